# revision 22
# baseline (speedup 1.0000x reference)
"""Bass/Trainium2 kernel for nn_ExtractModel (soft banded edit-distance vocab matcher).

Sharding: vocab axis V=1000 split 8 x 125 across NeuronCores (partition dim = vocab).

Design (v1 baseline: 123.5 us -> v5):
  - ext[b,s,i] = word_repr[b, min(s+i, L-1)] has only L=48 distinct columns per
    batch, so the cosine matmul runs over the 192 distinct (b,l) pairs plus 9
    replicated clamp columns (228 total) instead of all 1920 windowed columns.
    Each DP band cell (i,j) reads a SHIFTED slice of the matmul output - the
    window gather becomes an access pattern, not data movement.
  - bf16 matmul operands (4x PE vs fp32; dot error ~1e-3 against a 0.3 margin
    on the 0.05 match threshold).
  - h-space DP: h[i,j] = f[i,j] - (i+j). The ins/del "+1" transitions become
    "+0" and every boundary constant becomes 0, so each band cell needs only
    TENSOR_TENSOR min/add and TENSOR_SCALAR ops - these hit the DVE 16-bit
    fast path on HW (~210-260 ns for [125,192] fp16), unlike
    SCALAR_TENSOR_TENSOR (~350 ns, measured). The sub-path needs
    (h_sub + dij - 2); the "-2" is folded into the ACT bias:
    dpad = -1.5 - 0.5*dot.  Host adds (i+j) back.
  - Word + vocab ship as ONE packed bf16 dram tensor, split into two DMAs
    (word+vocab j0..3, then j4..9) so PE starts after ~55% of input bytes.
  - 8 PSUM banks, one j-group per bank, ACT releases banks. A zero-gap
    close/reopen of the same bank wedges the PE (NRT_EXEC_UNIT_UNRECOVERABLE,
    measured), so reopens only happen with multi-microsecond gaps.
  - ACT warmup op hides the one-time ACT_TABLE_LOAD (~1.3 us).
  - fp16 output DMA'd in 12 chunks of 3 band cells on the SP queue, overlapped
    with the DP. (Adding more DMA queues for the output slows every DVE op
    ~30% through SBUF contention - a net loss, measured.)
  (GpSimd/Pool cannot run TensorTensor/TensorScalar in this toolchain - the
  walrus engine check rejects them - so the DP stays entirely on DVE.)

Raw Bass (no TileContext); all cross-engine syncs are standalone wait_ge
instructions (walrus rejects >1 attached wait). Semaphores (kept to 6 - each
extra semaphore lengthens the fixed walrus init preamble):
  s_in  : +1 scratch memset, +16 input chunk A   (GpSimd -> ACT/PE)
  s_vb  : +16 input chunk B                      (GpSimd -> PE)
  s_pe  : per-j psum group finished              (PE -> ACT)
  s_act : dpad j-slices in SBUF                  (ACT -> DVE, PE bank release)
  s_dve : band cells finished                    (DVE -> 3-cell output chunks)
  s_out : output DMAs done

The reference's second DP table (not_viable init, all-BIG) provably yields
values >= 99.9 > MATCH_THRESH everywhere, so non-viable positions always score
exactly +/-0.0 and never match; constant BIG gives identical final outputs.
Out-of-band DP predecessors (>= BIG) never win the min and are dropped.
Shapes hardcoded per the problem spec.
"""

import numpy as np
import ml_dtypes

import concourse.bass as bass
import concourse.mybir as mybir
from concourse.bass_utils import run_bass_kernel_spmd

MSL = 10
MTL = 10
BIG = 99.9
MATCH_THRESH = 0.05
BS, L, D, V = 4, 48, 256, 1000
NCORES = 8
VC = V // NCORES          # 125 vocab words per core
M = BS * L                # 192 (b,s) positions
LP = L + MSL - 1          # 57 padded l columns (48 real + 9 clamp copies)
MP = BS * LP              # 228 matmul moving columns
KC = D // 128             # 2 contraction chunks
JA = 4                    # vocab j's in input chunk A
WCOL = KC * MP            # 456 word columns in the packed input
NCOL = WCOL + MTL * KC * VC   # 2956 packed input columns
CUTA = WCOL + JA * KC * VC    # 1456 = end of chunk A
F32 = mybir.dt.float32
F16 = mybir.dt.float16
BF16 = mybir.dt.bfloat16

# band cells of the edit-distance DP, in dependency order
BAND = [(i, j) for i in range(1, MSL + 1)
        for j in range(max(i - 2, 1), min(i + 2, MTL + 1))]
BAND_IDX = {c: n for n, c in enumerate(BAND)}
NCELL = len(BAND)         # 36
CHUNK = 3                 # band cells per output DMA chunk

_prog_cache = {}


def _build_program():
    nc = bass.Bass()
    inT = nc.dram_tensor("inT", [128, NCOL], BF16, kind="ExternalInput")
    fband = nc.dram_tensor("fband", [VC, NCELL, BS, L], F16, kind="ExternalOutput")

    import contextlib
    with contextlib.ExitStack() as ctx:
        ent = ctx.enter_context
        in_s = ent(nc.sbuf_tensor("in_s", [128, NCOL], BF16))
        dpad = ent(nc.sbuf_tensor("dpad", [VC, MTL, BS, LP], F16))
        fall = ent(nc.sbuf_tensor("fall", [VC, NCELL, BS, L], F16))
        tmpA = ent(nc.sbuf_tensor("tmpA", [VC, BS, L], F16))
        tmpB = ent(nc.sbuf_tensor("tmpB", [VC, BS, L], F16))
        warm = ent(nc.sbuf_tensor("warm", [VC, 2], F32))
        ps = [ent(nc.psum_tensor(f"ps{b}", [VC, BS, LP], F32)) for b in range(8)]
        s_in = ent(nc.semaphore("s_in"))
        s_vb = ent(nc.semaphore("s_vb"))
        s_pe = ent(nc.semaphore("s_pe"))
        s_act = ent(nc.semaphore("s_act"))
        s_dve = ent(nc.semaphore("s_dve"))
        s_out = ent(nc.semaphore("s_out"))
        s_oa = ent(nc.semaphore("s_oa"))
        s_ob = ent(nc.semaphore("s_ob"))

        def wordmv(kc):
            # word moving operand [128, 228] for contraction half kc
            return in_s[:, kc * MP:(kc + 1) * MP]

        def vocst(j, kc):
            # vocab stationary operand [128, 125] for (j, kc)
            off = WCOL + (j * KC + kc) * VC
            return in_s[:, off:off + VC]

        nq = NCELL // CHUNK  # 12 output chunks

        def out_chunks(engine, qs):
            for q in qs:
                engine.wait_ge(s_dve, CHUNK * (q + 1))
                engine.dma_start(fband[:, q * CHUNK:(q + 1) * CHUNK],
                                 fall[:, q * CHUNK:(q + 1) * CHUNK]
                                 ).then_inc(s_out, 16)

        with nc.Block() as block:

            @block.gpsimd
            def _(gpsimd):
                # scratch init for the ACT warmup (CoreSim rejects
                # uninitialized reads; HW is indifferent). The memset signals
                # via s_pe: a GpSimd (SWDGE) DMA semaphore update is
                # SET-semantics, so each gpsimd DMA needs a dedicated
                # semaphore that stays 0 until it completes.
                gpsimd.memset(tmpA[:, 0, 0:2], 0.0).then_inc(s_pe, 1)
                gpsimd.dma_start(in_s[:, 0:CUTA], inT[:, 0:CUTA]).then_inc(s_in, 16)
                gpsimd.dma_start(in_s[:, CUTA:NCOL], inT[:, CUTA:NCOL]
                                 ).then_inc(s_vb, 16)
                # tail output chunks on the GpSimd queue: the SP queue is
                # still draining its backlog when the DP finishes (SWDGE sem
                # updates are SET-semantics, hence one dedicated sem per DMA)
                gpsimd.wait_ge(s_dve, CHUNK * 11)
                gpsimd.dma_start(fband[:, 30:33], fall[:, 30:33]).then_inc(s_oa, 16)
                gpsimd.wait_ge(s_dve, CHUNK * 12)
                gpsimd.dma_start(fband[:, 33:36], fall[:, 33:36]).then_inc(s_ob, 16)
                gpsimd.wait_ge(s_oa, 16)
                gpsimd.wait_ge(s_ob, 16)

            @block.sync
            def _(sync):
                out_chunks(sync, range(nq - 2))
                sync.wait_ge(s_out, 16 * (nq - 2))

            @block.tensor
            def _(tensor):
                tensor.wait_ge(s_in, 16)
                for j in range(MTL):
                    if j == JA:
                        tensor.wait_ge(s_vb, 16)
                    if j >= 8:
                        tensor.wait_ge(s_act, j - 7)  # bank j%8 released by ACT
                    tensor.matmul(ps[j % 8][:], vocst(j, 0), wordmv(0),
                                  start=True, stop=False)
                    tensor.matmul(ps[j % 8][:], vocst(j, 1), wordmv(1),
                                  start=False, stop=True).then_inc(s_pe, 1)

            @block.scalar
            def _(scalar):
                # warmup: pull the one-time ACT table load off the critical path
                scalar.wait_ge(s_pe, 1)
                scalar.activation(warm[:], tmpA[:, 0, 0:2],
                                  mybir.ActivationFunctionType.Copy)
                for j in range(MTL):
                    scalar.wait_ge(s_pe, j + 2)
                    scalar.activation(
                        dpad[:, j], ps[j % 8][:],
                        mybir.ActivationFunctionType.Copy, bias=-1.5, scale=-0.5,
                    ).then_inc(s_act, 1)


            @block.vector
            def _(vector):
                Alu = mybir.AluOpType
                fmap = {}

                def pred(i, j):
                    # h-space: boundary rows/cols are all 0; out-of-band is BIG
                    if (i, j) in fmap:
                        return fmap[(i, j)]
                    if i == 0 or j == 0:
                        return 0.0
                    return BIG

                waited = 0
                for (i, j) in BAND:
                    dij = dpad[:, j - 1, :, i - 1:i - 1 + L]  # = diff - 2
                    if j > waited:
                        vector.wait_ge(s_act, j)
                        waited = j

                    sub_p = pred(i - 1, j - 1)
                    tens = [p for p in (pred(i - 1, j), pred(i, j - 1))
                            if not isinstance(p, float)]
                    n = BAND_IDX[(i, j)]
                    fcell = fall[:, n]
                    fmap[(i, j)] = fcell
                    tmps = [tmpA[:], tmpB[:]]
                    nops = 1 + len(tens)
                    k = 0

                    def out_of(k):
                        return fcell if k == nops - 1 else tmps[k]

                    if isinstance(sub_p, float):
                        # sub pred is the 0 boundary: min(dij' + 0, 0) also
                        # covers the (always present) 0-const ins/del pred
                        ins = vector.tensor_scalar(
                            out_of(k), dij, 0.0, 0.0, Alu.add, Alu.min)
                    else:
                        ins = vector.tensor_tensor(
                            out_of(k), sub_p, dij, Alu.add)
                    acc = out_of(k)
                    k += 1
                    for t in tens:
                        ins = vector.tensor_tensor(out_of(k), acc, t, Alu.min)
                        acc = out_of(k)
                        k += 1
                    ins.then_inc(s_dve, 1)

    return nc


def _prep_inputs(word_repr, vocab_repr):
    """Host prep: cosine pre-normalization, clamp-column replication, packing."""
    word_repr = np.asarray(word_repr, dtype=np.float32)
    vocab_repr = np.asarray(vocab_repr, dtype=np.float32)
    nx = np.sqrt((word_repr * word_repr).sum(-1, dtype=np.float32)) + np.float32(1e-8)
    wordn = word_repr / nx[..., None]                            # [bs,L,d]
    ny = np.sqrt((vocab_repr * vocab_repr).sum(-1, dtype=np.float32)) + np.float32(1e-8)
    vocn = vocab_repr / ny[..., None]                            # [V,MTL,d]

    # wpad[b, l', d]: l' 0..47 real, 48..56 copies of column 47 (window clamp)
    wpad = np.concatenate(
        [wordn, np.repeat(wordn[:, L - 1:L, :], MSL - 1, axis=1)], axis=1)
    # word block: [k, kc, b, l'] flattened to [128, 456]  (d = kc*128 + k)
    wblk = (wpad.transpose(2, 0, 1).reshape(KC, 128, BS, LP)
            .transpose(1, 0, 2, 3).reshape(128, WCOL))
    in_maps = []
    for c in range(NCORES):
        vs = vocn[c * VC:(c + 1) * VC]                           # [125,10,256]
        # vocab block: [k, j, kc, v] flattened to [128, 2500]
        vblk = (vs.transpose(2, 1, 0).reshape(KC, 128, MTL, VC)
                .transpose(1, 2, 0, 3).reshape(128, MTL * KC * VC))
        inT = np.ascontiguousarray(
            np.concatenate([wblk, vblk], axis=1)).astype(ml_dtypes.bfloat16)
        in_maps.append({"inT": inT})
    return in_maps


def kernel(word_repr, vocab_repr, lengths, vocab_length):
    lengths = np.asarray(lengths)
    vocab_length = np.asarray(vocab_length)
    in_maps = _prep_inputs(word_repr, vocab_repr)

    # ----- device: matmul + banded DP on 8 cores -----
    global _last_in_maps
    _last_in_maps = in_maps
    if "nc" not in _prog_cache:
        _prog_cache["nc"] = _build_program()
    res = run_bass_kernel_spmd(_prog_cache["nc"], in_maps, list(range(NCORES)))
    fb = np.stack([res.results[c]["fband"].reshape(VC, NCELL, M)
                   .transpose(1, 0, 2) for c in range(NCORES)]
                  ).astype(np.float32)                           # [8,36,125,192]
    global _last_fb
    _last_fb = fb

    # ----- host finish: gather at vocab_length, min over V, score, argmax -----
    f_full = np.full((MSL + 1, MTL + 1, NCORES, VC, M), BIG, dtype=np.float32)
    for n, (i, j) in enumerate(BAND):
        f_full[i, j] = fb[:, n] + np.float32(i + j)   # undo h-space shift
    vl = vocab_length.astype(np.int64)                           # [1000] in 1..10
    v_core = np.arange(V) // VC
    v_loc = np.arange(V) % VC
    # val2[e, v, m] = f[e+1, vl[v], v]
    val2 = f_full[np.arange(1, MSL + 1)[:, None], vl[None, :],
                  v_core[None, :], v_loc[None, :], :]            # [10,1000,192]
    value = val2.transpose(2, 0, 1).reshape(BS, L, MSL, V)

    viable = (np.arange(L)[:, None] + np.arange(MSL)[None, :])[None] \
        < lengths[:, None, None]
    value = np.where(viable[..., None], value, np.float32(BIG))

    best_value = value.min(axis=-1)
    matched_vocab = value.argmin(axis=-1)
    lens = vl[matched_vocab].astype(np.float32)
    matched = best_value < np.float32(MATCH_THRESH)
    score = lens * matched.astype(np.float32) * (np.float32(1.0) - best_value)

    sf = score.reshape(BS, -1)
    best_scores = sf.max(axis=-1)
    best_inds = sf.argmax(axis=-1).astype(np.int32)
    best_starts = best_inds // MSL
    best_ends = best_inds % MSL + best_starts
    matched_any = matched.reshape(BS, -1).any(axis=-1)
    return (best_scores.astype(np.float32), best_starts.astype(np.int32),
            best_ends.astype(np.int32), matched_any)


# revision 23
# speedup vs baseline: 1.0246x; 1.0246x over previous
"""Bass/Trainium2 kernel for nn_ExtractModel (soft banded edit-distance vocab matcher).

Sharding: vocab axis V=1000 split 8 x 125 across NeuronCores (partition dim = vocab).

Design (v1 baseline: 123.5 us -> v5):
  - ext[b,s,i] = word_repr[b, min(s+i, L-1)] has only L=48 distinct columns per
    batch, so the cosine matmul runs over the 192 distinct (b,l) pairs plus 9
    replicated clamp columns (228 total) instead of all 1920 windowed columns.
    Each DP band cell (i,j) reads a SHIFTED slice of the matmul output - the
    window gather becomes an access pattern, not data movement.
  - bf16 matmul operands (4x PE vs fp32; dot error ~1e-3 against a 0.3 margin
    on the 0.05 match threshold).
  - h-space DP: h[i,j] = f[i,j] - (i+j). The ins/del "+1" transitions become
    "+0" and every boundary constant becomes 0, so each band cell needs only
    TENSOR_TENSOR min/add and TENSOR_SCALAR ops - these hit the DVE 16-bit
    fast path on HW (~210-260 ns for [125,192] fp16), unlike
    SCALAR_TENSOR_TENSOR (~350 ns, measured). The sub-path needs
    (h_sub + dij - 2); the "-2" is folded into the ACT bias:
    dpad = -1.5 - 0.5*dot.  Host adds (i+j) back.
  - Word + vocab ship as ONE packed bf16 dram tensor, split into two DMAs
    (word+vocab j0..3, then j4..9) so PE starts after ~55% of input bytes.
  - 8 PSUM banks, one j-group per bank, ACT releases banks. A zero-gap
    close/reopen of the same bank wedges the PE (NRT_EXEC_UNIT_UNRECOVERABLE,
    measured), so reopens only happen with multi-microsecond gaps.
  - ACT warmup op hides the one-time ACT_TABLE_LOAD (~1.3 us).
  - fp16 output DMA'd in 12 chunks of 3 band cells on the SP queue, overlapped
    with the DP. (Adding more DMA queues for the output slows every DVE op
    ~30% through SBUF contention - a net loss, measured.)
  (GpSimd/Pool cannot run TensorTensor/TensorScalar in this toolchain - the
  walrus engine check rejects them - so the DP stays entirely on DVE.)

Raw Bass (no TileContext); all cross-engine syncs are standalone wait_ge
instructions (walrus rejects >1 attached wait). Semaphores (kept to 6 - each
extra semaphore lengthens the fixed walrus init preamble):
  s_in  : +1 scratch memset, +16 input chunk A   (GpSimd -> ACT/PE)
  s_vb  : +16 input chunk B                      (GpSimd -> PE)
  s_pe  : per-j psum group finished              (PE -> ACT)
  s_act : dpad j-slices in SBUF                  (ACT -> DVE, PE bank release)
  s_dve : band cells finished                    (DVE -> 3-cell output chunks)
  s_out : output DMAs done

The reference's second DP table (not_viable init, all-BIG) provably yields
values >= 99.9 > MATCH_THRESH everywhere, so non-viable positions always score
exactly +/-0.0 and never match; constant BIG gives identical final outputs.
Out-of-band DP predecessors (>= BIG) never win the min and are dropped.
Shapes hardcoded per the problem spec.
"""

import numpy as np
import ml_dtypes

import concourse.bass as bass
import concourse.mybir as mybir
from concourse.bass_utils import run_bass_kernel_spmd

MSL = 10
MTL = 10
BIG = 99.9
MATCH_THRESH = 0.05
BS, L, D, V = 4, 48, 256, 1000
NCORES = 8
VC = V // NCORES          # 125 vocab words per core
M = BS * L                # 192 (b,s) positions
LP = L + MSL - 1          # 57 padded l columns (48 real + 9 clamp copies)
MP = BS * LP              # 228 matmul moving columns
KC = D // 128             # 2 contraction chunks
JA = 4                    # vocab j's in input chunk A
WCOL = KC * MP            # 456 word columns in the packed input
NCOL = WCOL + MTL * KC * VC   # 2956 packed input columns
CUTA = WCOL + JA * KC * VC    # 1456 = end of chunk A
F32 = mybir.dt.float32
F16 = mybir.dt.float16
BF16 = mybir.dt.bfloat16

# band cells of the edit-distance DP, in dependency order
BAND = [(i, j) for i in range(1, MSL + 1)
        for j in range(max(i - 2, 1), min(i + 2, MTL + 1))]
BAND_IDX = {c: n for n, c in enumerate(BAND)}
NCELL = len(BAND)         # 36
CHUNK = 3                 # band cells per output DMA chunk

_prog_cache = {}


def _build_program():
    nc = bass.Bass()
    inT = nc.dram_tensor("inT", [128, NCOL], BF16, kind="ExternalInput")
    fband = nc.dram_tensor("fband", [VC, NCELL, BS, L], F16, kind="ExternalOutput")

    import contextlib
    with contextlib.ExitStack() as ctx:
        ent = ctx.enter_context
        in_s = ent(nc.sbuf_tensor("in_s", [128, NCOL], BF16))
        dpad = ent(nc.sbuf_tensor("dpad", [VC, MTL, BS, LP], F16))
        fall = ent(nc.sbuf_tensor("fall", [VC, NCELL, BS, L], F16))
        tmpA = ent(nc.sbuf_tensor("tmpA", [VC, BS, L], F16))
        tmpB = ent(nc.sbuf_tensor("tmpB", [VC, BS, L], F16))
        warm = ent(nc.sbuf_tensor("warm", [VC, 2], F32))
        ps = [ent(nc.psum_tensor(f"ps{b}", [VC, BS, LP], F32)) for b in range(8)]
        s_in = ent(nc.semaphore("s_in"))
        s_vb = ent(nc.semaphore("s_vb"))
        s_pe = ent(nc.semaphore("s_pe"))
        s_act = ent(nc.semaphore("s_act"))
        s_dve = ent(nc.semaphore("s_dve"))
        s_out = ent(nc.semaphore("s_out"))

        def wordmv(kc):
            # word moving operand [128, 228] for contraction half kc
            return in_s[:, kc * MP:(kc + 1) * MP]

        def vocst(j, kc):
            # vocab stationary operand [128, 125] for (j, kc)
            off = WCOL + (j * KC + kc) * VC
            return in_s[:, off:off + VC]

        nq = NCELL // CHUNK  # 12 output chunks

        def out_chunks(engine, qs):
            for q in qs:
                engine.wait_ge(s_dve, CHUNK * (q + 1))
                engine.dma_start(fband[:, q * CHUNK:(q + 1) * CHUNK],
                                 fall[:, q * CHUNK:(q + 1) * CHUNK]
                                 ).then_inc(s_out, 16)

        with nc.Block() as block:

            @block.gpsimd
            def _(gpsimd):
                # scratch init for the ACT warmup (CoreSim rejects
                # uninitialized reads; HW is indifferent). The memset signals
                # via s_pe: a GpSimd (SWDGE) DMA semaphore update is
                # SET-semantics, so each gpsimd DMA needs a dedicated
                # semaphore that stays 0 until it completes.
                gpsimd.memset(tmpA[:, 0, 0:2], 0.0).then_inc(s_pe, 1)
                gpsimd.dma_start(in_s[:, 0:CUTA], inT[:, 0:CUTA]).then_inc(s_in, 16)
                gpsimd.dma_start(in_s[:, CUTA:NCOL], inT[:, CUTA:NCOL]
                                 ).then_inc(s_vb, 16)

            @block.sync
            def _(sync):
                out_chunks(sync, range(nq))
                sync.wait_ge(s_out, 16 * nq)

            @block.tensor
            def _(tensor):
                tensor.wait_ge(s_in, 16)
                for j in range(MTL):
                    if j == JA:
                        tensor.wait_ge(s_vb, 16)
                    if j >= 8:
                        tensor.wait_ge(s_act, j - 7)  # bank j%8 released by ACT
                    tensor.matmul(ps[j % 8][:], vocst(j, 0), wordmv(0),
                                  start=True, stop=False)
                    tensor.matmul(ps[j % 8][:], vocst(j, 1), wordmv(1),
                                  start=False, stop=True).then_inc(s_pe, 1)

            @block.scalar
            def _(scalar):
                # warmup: pull the one-time ACT table load off the critical path
                scalar.wait_ge(s_pe, 1)
                scalar.activation(warm[:], tmpA[:, 0, 0:2],
                                  mybir.ActivationFunctionType.Copy)
                for j in range(MTL):
                    scalar.wait_ge(s_pe, j + 2)
                    scalar.activation(
                        dpad[:, j], ps[j % 8][:],
                        mybir.ActivationFunctionType.Copy, bias=-1.5, scale=-0.5,
                    ).then_inc(s_act, 1)


            @block.vector
            def _(vector):
                Alu = mybir.AluOpType
                fmap = {}

                def pred(i, j):
                    # h-space: boundary rows/cols are all 0; out-of-band is BIG
                    if (i, j) in fmap:
                        return fmap[(i, j)]
                    if i == 0 or j == 0:
                        return 0.0
                    return BIG

                waited = 0
                for (i, j) in BAND:
                    dij = dpad[:, j - 1, :, i - 1:i - 1 + L]  # = diff - 2
                    if j > waited:
                        vector.wait_ge(s_act, j)
                        waited = j

                    sub_p = pred(i - 1, j - 1)
                    tens = [p for p in (pred(i - 1, j), pred(i, j - 1))
                            if not isinstance(p, float)]
                    n = BAND_IDX[(i, j)]
                    fcell = fall[:, n]
                    fmap[(i, j)] = fcell
                    tmps = [tmpA[:], tmpB[:]]
                    nops = 1 + len(tens)
                    k = 0

                    def out_of(k):
                        return fcell if k == nops - 1 else tmps[k]

                    if isinstance(sub_p, float):
                        # sub pred is the 0 boundary: min(dij' + 0, 0) also
                        # covers the (always present) 0-const ins/del pred
                        ins = vector.tensor_scalar(
                            out_of(k), dij, 0.0, 0.0, Alu.add, Alu.min)
                    else:
                        ins = vector.tensor_tensor(
                            out_of(k), sub_p, dij, Alu.add)
                    acc = out_of(k)
                    k += 1
                    for t in tens:
                        ins = vector.tensor_tensor(out_of(k), acc, t, Alu.min)
                        acc = out_of(k)
                        k += 1
                    ins.then_inc(s_dve, 1)

    return nc


def _prep_inputs(word_repr, vocab_repr):
    """Host prep: cosine pre-normalization, clamp-column replication, packing."""
    word_repr = np.asarray(word_repr, dtype=np.float32)
    vocab_repr = np.asarray(vocab_repr, dtype=np.float32)
    nx = np.sqrt((word_repr * word_repr).sum(-1, dtype=np.float32)) + np.float32(1e-8)
    wordn = word_repr / nx[..., None]                            # [bs,L,d]
    ny = np.sqrt((vocab_repr * vocab_repr).sum(-1, dtype=np.float32)) + np.float32(1e-8)
    vocn = vocab_repr / ny[..., None]                            # [V,MTL,d]

    # wpad[b, l', d]: l' 0..47 real, 48..56 copies of column 47 (window clamp)
    wpad = np.concatenate(
        [wordn, np.repeat(wordn[:, L - 1:L, :], MSL - 1, axis=1)], axis=1)
    # word block: [k, kc, b, l'] flattened to [128, 456]  (d = kc*128 + k)
    wblk = (wpad.transpose(2, 0, 1).reshape(KC, 128, BS, LP)
            .transpose(1, 0, 2, 3).reshape(128, WCOL))
    in_maps = []
    for c in range(NCORES):
        vs = vocn[c * VC:(c + 1) * VC]                           # [125,10,256]
        # vocab block: [k, j, kc, v] flattened to [128, 2500]
        vblk = (vs.transpose(2, 1, 0).reshape(KC, 128, MTL, VC)
                .transpose(1, 2, 0, 3).reshape(128, MTL * KC * VC))
        inT = np.ascontiguousarray(
            np.concatenate([wblk, vblk], axis=1)).astype(ml_dtypes.bfloat16)
        in_maps.append({"inT": inT})
    return in_maps


def kernel(word_repr, vocab_repr, lengths, vocab_length):
    lengths = np.asarray(lengths)
    vocab_length = np.asarray(vocab_length)
    in_maps = _prep_inputs(word_repr, vocab_repr)

    # ----- device: matmul + banded DP on 8 cores -----
    global _last_in_maps
    _last_in_maps = in_maps
    if "nc" not in _prog_cache:
        _prog_cache["nc"] = _build_program()
    res = run_bass_kernel_spmd(_prog_cache["nc"], in_maps, list(range(NCORES)))
    fb = np.stack([res.results[c]["fband"].reshape(VC, NCELL, M)
                   .transpose(1, 0, 2) for c in range(NCORES)]
                  ).astype(np.float32)                           # [8,36,125,192]
    global _last_fb
    _last_fb = fb

    # ----- host finish: gather at vocab_length, min over V, score, argmax -----
    f_full = np.full((MSL + 1, MTL + 1, NCORES, VC, M), BIG, dtype=np.float32)
    for n, (i, j) in enumerate(BAND):
        f_full[i, j] = fb[:, n] + np.float32(i + j)   # undo h-space shift
    vl = vocab_length.astype(np.int64)                           # [1000] in 1..10
    v_core = np.arange(V) // VC
    v_loc = np.arange(V) % VC
    # val2[e, v, m] = f[e+1, vl[v], v]
    val2 = f_full[np.arange(1, MSL + 1)[:, None], vl[None, :],
                  v_core[None, :], v_loc[None, :], :]            # [10,1000,192]
    value = val2.transpose(2, 0, 1).reshape(BS, L, MSL, V)

    viable = (np.arange(L)[:, None] + np.arange(MSL)[None, :])[None] \
        < lengths[:, None, None]
    value = np.where(viable[..., None], value, np.float32(BIG))

    best_value = value.min(axis=-1)
    matched_vocab = value.argmin(axis=-1)
    lens = vl[matched_vocab].astype(np.float32)
    matched = best_value < np.float32(MATCH_THRESH)
    score = lens * matched.astype(np.float32) * (np.float32(1.0) - best_value)

    sf = score.reshape(BS, -1)
    best_scores = sf.max(axis=-1)
    best_inds = sf.argmax(axis=-1).astype(np.int32)
    best_starts = best_inds // MSL
    best_ends = best_inds % MSL + best_starts
    matched_any = matched.reshape(BS, -1).any(axis=-1)
    return (best_scores.astype(np.float32), best_starts.astype(np.int32),
            best_ends.astype(np.int32), matched_any)
